# revision 10
# baseline (speedup 1.0000x reference)
"""Cosine-similarity kernel for trn2: out = l2norm_rows(x) @ l2norm_rows(W).

x: [65536, 512] f32, W: [512, 462] f32 -> out: [65536, 462] f32.

Strategy (data-parallel over 8 cores, batch-sharded x, replicated W):
  The host hands each core x^T for its batch shard (layout marshaling
  only) so the contraction dim (in_dim) lands on SBUF partitions.

  Per core (8192 batch rows), per group of 1024 rows:
  - GEMM in NATURAL output layout: stationary = x^T tile [128K, 128b]
    (a direct slice of the x^T SBUF tile, no transpose), moving =
    normalized W chunk [128K, 462o], f32r.  PSUM out [128b, 462o].
  - Row rsqrt-sumsq ("s-chain") is software-pipelined ONE GROUP AHEAD
    of the GEMM so evictions never wait on it: squares (ACT/DVE
    split), ones-matmul partition reduce -> ssq [1,1024], SBUF->SBUF
    DMA shuffle to [8,128] (scalar HWDGE queue), one eye8 matmul ->
    [128,8] partition-major, sqrt(+eps) on ACT, reciprocal on DVE.
  - Eviction fuses the normalize: Copy-with-per-partition-scale (ACT
    for even b-tiles, tensor_scalar_mul on DVE for odd).
  - Outputs stored in natural layout via gpsimd SWDGE in quarter-group
    chunks so the store drains early; input x DMAs own the sync HWDGE
    queue, s-shuffles + W/eye8 the scalar HWDGE queue.
"""

from contextlib import ExitStack

import numpy as np

import concourse.bass as bass
import concourse.mybir as mybir
import concourse.tile as tile
from concourse import bacc, bass_utils
from concourse.bass import ds

N_CORES = 8
B = 65536
B_PER = B // N_CORES          # 8192 batch rows per core
IN_DIM = 512
OUT_DIM = 462
EPS = 1e-12
P = 128
KC = IN_DIM // P              # 4 contraction chunks
GROUP_COLS = 1024             # batch rows per group (2 MB in)
JT = GROUP_COLS // P          # 8 b-tiles of 128 rows per group
N_GROUPS = B_PER // GROUP_COLS

F32 = mybir.dt.float32
F32R = mybir.dt.float32r


def _build_bass():
    nc = bacc.Bacc("TRN2", debug=False, num_devices=N_CORES)
    xt_d = nc.dram_tensor("xt", [IN_DIM, B_PER], F32R, kind="ExternalInput").ap()
    w_d = nc.dram_tensor("w", [IN_DIM, OUT_DIM], F32, kind="ExternalInput").ap()
    o_d = nc.dram_tensor("o", [B_PER, OUT_DIM], F32, kind="ExternalOutput").ap()
    eye_d = nc.dram_tensor("eye8", [8, 8], F32, kind="ExternalInput").ap()

    with ExitStack() as ctx:
        tc = ctx.enter_context(tile.TileContext(nc))

        singles = ctx.enter_context(tc.tile_pool(name="singles", bufs=1))
        xpool = ctx.enter_context(tc.tile_pool(name="xin", bufs=3))
        sqpool = ctx.enter_context(tc.tile_pool(name="sq", bufs=2))
        opool = ctx.enter_context(tc.tile_pool(name="oout", bufs=2))
        stats = ctx.enter_context(tc.tile_pool(name="stats", bufs=3))
        psum_o = ctx.enter_context(tc.tile_pool(name="psum_o", bufs=3, space="PSUM"))
        psum_s = ctx.enter_context(tc.tile_pool(name="psum_s", bufs=3, space="PSUM"))
        psum_t = ctx.enter_context(tc.tile_pool(name="psum_t", bufs=2, space="PSUM"))

        zero_bias = singles.tile([P, 1], F32)
        nc.vector.memset(zero_bias, 0.0)
        ones_f = singles.tile([P, 1], F32)
        nc.vector.memset(ones_f, 1.0)
        ones_k = singles.tile([P, 1], F32R)   # reduce-over-partitions stationary
        nc.vector.tensor_copy(out=ones_k, in_=ones_f)
        eps_bias = singles.tile([P, 1], F32)
        nc.vector.memset(eps_bias, EPS)
        eye8 = singles.tile([8, 8], F32)      # transpose moving operand
        nc.scalar.dma_start(eye8, eye_d)

        # ---- W normalization (once; scalar queue so x0 owns sync) ----
        w_sb = singles.tile([P, KC, OUT_DIM], F32)
        nc.scalar.dma_start(w_sb, w_d.rearrange("(c p) o -> p c o", p=P))
        wsq = singles.tile([P, KC, OUT_DIM], F32)  # scratch squares
        wssq = singles.tile([P, KC], F32)
        for c in range(KC):
            nc.scalar.activation(
                out=wsq[:, c, :],
                in_=w_sb[:, c, :],
                func=mybir.ActivationFunctionType.Square,
                bias=zero_bias,
                accum_out=wssq[:, c : c + 1],
            )
        nc.vector.tensor_scalar_max(wssq, wssq, EPS)
        nc.scalar.activation(
            out=wssq, in_=wssq, func=mybir.ActivationFunctionType.Sqrt, bias=zero_bias
        )
        wrs = singles.tile([P, KC], F32)
        nc.vector.reciprocal(wrs, wssq)
        # f32r so the PE matmul runs at 1 cycle/row; producer rounds to f32r
        wn_sb = singles.tile([P, KC, OUT_DIM], F32R)
        for c in range(KC):
            nc.vector.tensor_scalar_mul(wn_sb[:, c, :], w_sb[:, c, :], wrs[:, c : c + 1])

        xt_v = xt_d.rearrange("(c p) b -> p c b", p=P)  # [128, KC, B_PER]
        x_tiles = {}
        s_mid = {}   # (s8 tile) between chain parts
        s_cols = {}

        def emit_xdma(g):
            x_sb = xpool.tile([P, KC, GROUP_COLS], F32R)
            nc.sync.dma_start(x_sb, xt_v[:, :, ds(g * GROUP_COLS, GROUP_COLS)])
            x_tiles[g] = x_sb

        def emit_schain_a(g):
            """squares + partition reduce + evict + shuffle DMA."""
            x_sb = x_tiles[g]
            xsq = sqpool.tile([P, KC, GROUP_COLS], F32R)
            nc.scalar.activation(
                out=xsq[:, 0:2, :],
                in_=x_sb[:, 0:2, :],
                func=mybir.ActivationFunctionType.Square,
                bias=zero_bias,
            )
            nc.vector.tensor_mul(xsq[:, 2:4, :], x_sb[:, 2:4, :], x_sb[:, 2:4, :])
            s_row = stats.tile([1, GROUP_COLS], F32)
            for h in range(GROUP_COLS // 512):
                ps_ssq = psum_s.tile([1, 512], F32)
                for c in range(KC):
                    nc.tensor.matmul(
                        ps_ssq,
                        lhsT=ones_k[:, :],
                        rhs=xsq[:, c, ds(h * 512, 512)],
                        start=(c == 0),
                        stop=(c == KC - 1),
                    )
                nc.vector.tensor_copy(out=s_row[:, ds(h * 512, 512)], in_=ps_ssq)
            s8 = stats.tile([8, P], F32)
            nc.scalar.dma_start(s8, s_row)
            s_mid[g] = s8

        def emit_schain_b(g):
            """eye8 transpose matmul + sqrt + reciprocal."""
            s8 = s_mid.pop(g)
            ps_s = psum_t.tile([P, JT], F32)
            nc.tensor.matmul(ps_s, lhsT=s8, rhs=eye8)
            sq_s = stats.tile([P, JT], F32)
            nc.scalar.activation(
                out=sq_s,
                in_=ps_s,
                func=mybir.ActivationFunctionType.Sqrt,
                bias=eps_bias,
            )
            s_col = stats.tile([P, JT], F32)
            nc.vector.reciprocal(s_col, sq_s)
            s_cols[g] = s_col

        def out_dma(ot, b_off, jh):
            """store [128, 2, OUT_DIM] quarter: rows b_off + jh*(2*2*P) + {0,2}*P + p"""
            dst = bass.AP(
                tensor=o_d.tensor,
                offset=(b_off + jh * 4 * P) * OUT_DIM,
                ap=[[OUT_DIM, P], [2 * P * OUT_DIM, 2], [1, OUT_DIM]],
            )
            nc.gpsimd.dma_start(dst, ot[:, ds(jh * 2, 2), :])

        # ---- prologue ----
        for g in range(min(3, N_GROUPS)):
            emit_xdma(g)
        emit_schain_a(0)
        emit_schain_b(0)

        # ---- steady-state loop: GEMM(g) runs with s_col(g) precomputed ----
        for g in range(N_GROUPS):
            b0 = g * GROUP_COLS
            x_sb = x_tiles[g]
            s_col = s_cols.pop(g)
            if g + 3 < N_GROUPS:
                emit_xdma(g + 3)
            if g + 1 < N_GROUPS:
                emit_schain_a(g + 1)

            ot_a = opool.tile([P, JT // 2, OUT_DIM], F32)  # even j  (ACT evict)
            ot_b = opool.tile([P, JT // 2, OUT_DIM], F32)  # odd j   (DVE evict)
            for j in range(JT):
                po = psum_o.tile([P, OUT_DIM], F32)
                for c in range(KC):
                    nc.tensor.matmul(
                        po,
                        lhsT=x_sb[:, c, ds(j * P, P)],
                        rhs=wn_sb[:, c, :],
                        start=(c == 0),
                        stop=(c == KC - 1),
                    )
                # fused normalize: per-partition scale while evicting PSUM
                if j % 2 == 0:
                    nc.scalar.activation(
                        out=ot_a[:, j // 2, :],
                        in_=po,
                        func=mybir.ActivationFunctionType.Copy,
                        scale=s_col[:, j : j + 1],
                    )
                else:
                    nc.vector.tensor_scalar_mul(
                        ot_b[:, j // 2, :], po, s_col[:, j : j + 1]
                    )
                if j == 3:
                    # hide the s-shuffle latency: flip next group's ssq
                    # mid-GEMM, then store the first quarter outputs
                    if g + 1 < N_GROUPS:
                        emit_schain_b(g + 1)
                    out_dma(ot_a, b0, 0)
                    out_dma(ot_b, b0 + P, 0)
            out_dma(ot_a, b0, 1)
            out_dma(ot_b, b0 + P, 1)
            del x_tiles[g]

    nc.compile()
    return nc


_NC_CACHE = None
LAST_RESULTS = None  # BassKernelResults of the most recent run (for profiling)


def kernel(x: np.ndarray, W: np.ndarray) -> np.ndarray:
    global _NC_CACHE, LAST_RESULTS
    if _NC_CACHE is None:
        _NC_CACHE = _build_bass()
    nc = _NC_CACHE

    x = np.asarray(x, dtype=np.float32)
    W = np.ascontiguousarray(np.asarray(W, dtype=np.float32))
    in_maps = []
    for i in range(N_CORES):
        shard = np.ascontiguousarray(x[i * B_PER : (i + 1) * B_PER].T)
        in_maps.append({"xt": shard, "w": W, "eye8": np.eye(8, dtype=np.float32)})
    res = bass_utils.run_bass_kernel_spmd(nc, in_maps, core_ids=list(range(N_CORES)))
    LAST_RESULTS = res
    out = np.concatenate([np.asarray(r["o"]) for r in res.results], axis=0)
    return out


# revision 11
# speedup vs baseline: 1.1356x; 1.1356x over previous
"""Cosine-similarity kernel for trn2: out = l2norm_rows(x) @ l2norm_rows(W).

x: [65536, 512] f32, W: [512, 462] f32 -> out: [65536, 462] f32.

Strategy (data-parallel over 8 cores, batch-sharded x, replicated W):
  The host hands each core x^T for its batch shard, laid out so that
  (a) each group's input is one 8 KB-contiguous line per partition and
  (b) batch rows are permuted within each 512-row window (row 4p+j on
  tile j, partition p) so the OUTPUT store coalesces four consecutive
  DRAM rows into one 7392 B partition line (big-descriptor stores).

  Per core (8192 batch rows), per group of 512 rows:
  - GEMM in natural output layout: stationary = x^T tile [128K, 128b]
    (direct SBUF slice, no transpose), moving = normalized W chunk
    [128K, 462o], f32r.  PSUM out [128b, 462o].
  - Row rsqrt-sumsq ("s-chain") runs TWO GROUPS AHEAD of the GEMM so
    evictions never wait: squares (ACT/DVE split), ones-matmul
    partition reduce -> ssq [1,512], SBUF->SBUF DMA shuffle to
    [4,128] (scalar HWDGE queue), one eye4 matmul -> [128,4]
    partition-major, sqrt(+eps) on ACT, reciprocal on DVE.
  - Eviction fuses the normalize via per-partition scale; engine
    alternates per group (ACT / DVE) to balance load.
  - Outputs stored via gpsimd SWDGE; inputs own the sync HWDGE queue,
    s-shuffles + W/eye the scalar HWDGE queue.
"""

from contextlib import ExitStack

import numpy as np

import concourse.bass as bass
import concourse.mybir as mybir
import concourse.tile as tile
from concourse import bacc, bass_utils
from concourse.bass import ds

N_CORES = 8
B = 65536
B_PER = B // N_CORES          # 8192 batch rows per core
IN_DIM = 512
OUT_DIM = 462
EPS = 1e-12
P = 128
KC = IN_DIM // P              # 4 contraction chunks
G = 512                       # batch rows per group (1 MB in)
JT = G // P                   # 4 b-tiles of 128 rows per group
N_GROUPS = B_PER // G         # 16

F32 = mybir.dt.float32
F32R = mybir.dt.float32r


def _build_bass():
    nc = bacc.Bacc("TRN2", debug=False, num_devices=N_CORES)
    # [p, g, c, b] layout: one 8KB line per partition per group
    xt_d = nc.dram_tensor("xt", [P, N_GROUPS * KC * G], F32R, kind="ExternalInput").ap()
    w_d = nc.dram_tensor("w", [IN_DIM, OUT_DIM], F32, kind="ExternalInput").ap()
    o_d = nc.dram_tensor("o", [B_PER, OUT_DIM], F32, kind="ExternalOutput").ap()
    eye_d = nc.dram_tensor("eye4", [JT, JT], F32, kind="ExternalInput").ap()

    with ExitStack() as ctx:
        tc = ctx.enter_context(tile.TileContext(nc))

        singles = ctx.enter_context(tc.tile_pool(name="singles", bufs=1))
        xpool = ctx.enter_context(tc.tile_pool(name="xin", bufs=5))
        sqpool = ctx.enter_context(tc.tile_pool(name="sq", bufs=2))
        opool = ctx.enter_context(tc.tile_pool(name="oout", bufs=3))
        stats = ctx.enter_context(tc.tile_pool(name="stats", bufs=4))
        psum_o = ctx.enter_context(tc.tile_pool(name="psum_o", bufs=4, space="PSUM"))
        psum_s = ctx.enter_context(tc.tile_pool(name="psum_s", bufs=2, space="PSUM"))
        psum_t = ctx.enter_context(tc.tile_pool(name="psum_t", bufs=2, space="PSUM"))

        zero_bias = singles.tile([P, 1], F32)
        nc.vector.memset(zero_bias, 0.0)
        ones_f = singles.tile([P, 1], F32)
        nc.vector.memset(ones_f, 1.0)
        ones_k = singles.tile([P, 1], F32R)   # reduce-over-partitions stationary
        nc.vector.tensor_copy(out=ones_k, in_=ones_f)
        eps_bias = singles.tile([P, 1], F32)
        nc.vector.memset(eps_bias, EPS)
        eye4 = singles.tile([JT, JT], F32)    # transpose moving operand
        nc.scalar.dma_start(eye4, eye_d)

        # ---- W normalization (once; scalar queue so x0 owns sync) ----
        w_sb = singles.tile([P, KC, OUT_DIM], F32)
        nc.scalar.dma_start(w_sb, w_d.rearrange("(c p) o -> p c o", p=P))
        wsq = singles.tile([P, KC, OUT_DIM], F32)  # scratch squares
        wssq = singles.tile([P, KC], F32)
        for c in range(KC):
            nc.scalar.activation(
                out=wsq[:, c, :],
                in_=w_sb[:, c, :],
                func=mybir.ActivationFunctionType.Square,
                bias=zero_bias,
                accum_out=wssq[:, c : c + 1],
            )
        nc.vector.tensor_scalar_max(wssq, wssq, EPS)
        nc.scalar.activation(
            out=wssq, in_=wssq, func=mybir.ActivationFunctionType.Sqrt, bias=zero_bias
        )
        wrs = singles.tile([P, KC], F32)
        nc.vector.reciprocal(wrs, wssq)
        # f32r so the PE matmul runs at 1 cycle/row; producer rounds to f32r
        wn_sb = singles.tile([P, KC, OUT_DIM], F32R)
        for c in range(KC):
            nc.vector.tensor_scalar_mul(wn_sb[:, c, :], w_sb[:, c, :], wrs[:, c : c + 1])

        x_tiles = {}
        s_mid = {}
        s_cols = {}

        def emit_xdma(g):
            x_sb = xpool.tile([P, KC, G], F32R)
            nc.sync.dma_start(x_sb, xt_d[:, ds(g * KC * G, KC * G)])
            x_tiles[g] = x_sb

        def emit_schain_a(g):
            """squares + partition reduce + evict + shuffle DMA."""
            x_sb = x_tiles[g]
            xsq = sqpool.tile([P, KC, G], F32R)
            nc.scalar.activation(
                out=xsq[:, 0:2, :],
                in_=x_sb[:, 0:2, :],
                func=mybir.ActivationFunctionType.Square,
                bias=zero_bias,
            )
            nc.vector.tensor_mul(xsq[:, 2:4, :], x_sb[:, 2:4, :], x_sb[:, 2:4, :])
            ps_ssq = psum_s.tile([1, G], F32)
            for c in range(KC):
                nc.tensor.matmul(
                    ps_ssq,
                    lhsT=ones_k[:, :],
                    rhs=xsq[:, c, :],
                    start=(c == 0),
                    stop=(c == KC - 1),
                )
            s_row = stats.tile([1, G], F32)
            nc.vector.tensor_copy(out=s_row, in_=ps_ssq)
            s4 = stats.tile([JT, P], F32)
            nc.scalar.dma_start(s4, s_row)
            s_mid[g] = s4

        def emit_schain_b(g):
            """eye4 transpose matmul + sqrt + reciprocal."""
            s4 = s_mid.pop(g)
            ps_s = psum_t.tile([P, JT], F32)
            nc.tensor.matmul(ps_s, lhsT=s4, rhs=eye4)
            sq_s = stats.tile([P, JT], F32)
            nc.scalar.activation(
                out=sq_s,
                in_=ps_s,
                func=mybir.ActivationFunctionType.Sqrt,
                bias=eps_bias,
            )
            s_col = stats.tile([P, JT], F32)
            nc.vector.reciprocal(s_col, sq_s)
            s_cols[g] = s_col

        def emit_gemm_j(g, j, ot, s_col):
            po = psum_o.tile([P, OUT_DIM], F32)
            x_sb = x_tiles[g]
            for c in range(KC):
                nc.tensor.matmul(
                    po,
                    lhsT=x_sb[:, c, ds(j * P, P)],
                    rhs=wn_sb[:, c, :],
                    start=(c == 0),
                    stop=(c == KC - 1),
                )
            # fused normalize: per-partition scale while evicting PSUM;
            # engine alternates per group to balance ACT/DVE
            if g % 2 == 0:
                nc.scalar.activation(
                    out=ot[:, j, :],
                    in_=po,
                    func=mybir.ActivationFunctionType.Copy,
                    scale=s_col[:, j : j + 1],
                )
            else:
                nc.vector.tensor_scalar_mul(ot[:, j, :], po, s_col[:, j : j + 1])

        # ---- prologue ----
        for g in range(min(4, N_GROUPS)):
            emit_xdma(g)
        emit_schain_a(0)
        emit_schain_a(1)
        emit_schain_b(0)

        # ---- steady-state: GEMM(g) with s_col(g) precomputed; s-chain
        # runs two groups ahead (a: g+2, b: g+1) ----
        for g in range(N_GROUPS):
            s_col = s_cols.pop(g)
            if g + 4 < N_GROUPS:
                emit_xdma(g + 4)
            ot = opool.tile([P, JT, OUT_DIM], F32)
            emit_gemm_j(g, 0, ot, s_col)
            emit_gemm_j(g, 1, ot, s_col)
            if g + 2 < N_GROUPS:
                emit_schain_a(g + 2)
            emit_gemm_j(g, 2, ot, s_col)
            emit_gemm_j(g, 3, ot, s_col)
            if g + 1 < N_GROUPS:
                emit_schain_b(g + 1)
            # store: DRAM row = g*512 + 4p + j -> one 7392B line/partition
            dst = bass.AP(
                tensor=o_d.tensor,
                offset=g * G * OUT_DIM,
                ap=[[JT * OUT_DIM, P], [OUT_DIM, JT], [1, OUT_DIM]],
            )
            nc.gpsimd.dma_start(dst, ot)
            del x_tiles[g]

    nc.compile()
    return nc


_NC_CACHE = None
LAST_RESULTS = None  # BassKernelResults of the most recent run (for profiling)

# within each 512-row window: local column i <-> global row 4*(i%128) + i//128
_PERM = 4 * (np.arange(G) % P) + np.arange(G) // P


def kernel(x: np.ndarray, W: np.ndarray) -> np.ndarray:
    global _NC_CACHE, LAST_RESULTS
    if _NC_CACHE is None:
        _NC_CACHE = _build_bass()
    nc = _NC_CACHE

    x = np.asarray(x, dtype=np.float32)
    W = np.ascontiguousarray(np.asarray(W, dtype=np.float32))
    cols = np.arange(N_GROUPS)[:, None] * G + _PERM[None, :]   # [16, 512]
    in_maps = []
    for i in range(N_CORES):
        sT = x[i * B_PER : (i + 1) * B_PER].T                  # [512, 8192]
        tmp = sT[:, cols]                                      # [512, 16, 512]
        tmp = tmp.reshape(KC, P, N_GROUPS, G).transpose(1, 2, 0, 3)
        xt = np.ascontiguousarray(tmp.reshape(P, N_GROUPS * KC * G))
        in_maps.append({"xt": xt, "w": W, "eye4": np.eye(JT, dtype=np.float32)})
    res = bass_utils.run_bass_kernel_spmd(nc, in_maps, core_ids=list(range(N_CORES)))
    LAST_RESULTS = res
    out = np.concatenate([np.asarray(r["o"]) for r in res.results], axis=0)
    return out


# revision 16
# speedup vs baseline: 1.2652x; 1.1141x over previous
"""Cosine-similarity kernel for trn2: out = l2norm_rows(x) @ l2norm_rows(W).

x: [65536, 512] f32, W: [512, 462] f32 -> out: [65536, 462] f32.

Strategy (data-parallel over 8 cores, batch-sharded x, replicated W):
  The host hands each core x^T for its batch shard, laid out so that
  (a) each group's input is one 8 KB-contiguous line per partition and
  (b) batch rows are permuted within each 512-row window (row 4p+j on
  tile j, partition p) so the OUTPUT store coalesces four consecutive
  DRAM rows into one 7392 B partition line (big-descriptor stores).

  Per core (8192 batch rows), per group of 512 rows:
  - GEMM in natural output layout: stationary = x^T tile [128K, 128b]
    (direct SBUF slice, no transpose), moving = normalized W chunk
    [128K, 462o], f32r.  PSUM out [128b, 462o].
  - Row rsqrt-sumsq ("s-chain") runs TWO GROUPS AHEAD of the GEMM so
    evictions never wait: squares (ACT/DVE split), ones-matmul
    partition reduce -> ssq [1,512], SBUF->SBUF DMA shuffle to
    [4,128] (scalar HWDGE queue), one eye4 matmul -> [128,4]
    partition-major, sqrt(+eps) on ACT, reciprocal on DVE.
  - Eviction fuses the normalize via per-partition scale; engine
    alternates per group (ACT / DVE) to balance load.
  - Outputs stored via gpsimd SWDGE; inputs own the sync HWDGE queue,
    s-shuffles + W/eye the scalar HWDGE queue.
"""

from contextlib import ExitStack

import numpy as np

import concourse.bass as bass
import concourse.mybir as mybir
import concourse.tile as tile
from concourse import bacc, bass_utils
from concourse.bass import ds

N_CORES = 8
B = 65536
B_PER = B // N_CORES          # 8192 batch rows per core
IN_DIM = 512
OUT_DIM = 462
EPS = 1e-12
P = 128
KC = IN_DIM // P              # 4 contraction chunks
G = 512                       # batch rows per group (1 MB in)
JT = G // P                   # 4 b-tiles of 128 rows per group
N_GROUPS = B_PER // G         # 16

F32 = mybir.dt.float32
F32R = mybir.dt.float32r


def _build_bass():
    nc = bacc.Bacc("TRN2", debug=False, num_devices=N_CORES)
    # [p, g, c, b] layout: one 8KB line per partition per group
    xt_d = nc.dram_tensor("xt", [P, N_GROUPS * KC * G], F32R, kind="ExternalInput").ap()
    # [p, c, o] layout: one contiguous 7392B line per partition
    w_d = nc.dram_tensor("w", [P, KC * OUT_DIM], F32, kind="ExternalInput").ap()
    o_d = nc.dram_tensor("o", [B_PER, OUT_DIM], F32, kind="ExternalOutput").ap()
    eye_d = nc.dram_tensor("eye4", [JT, JT], F32, kind="ExternalInput").ap()

    with ExitStack() as ctx:
        tc = ctx.enter_context(tile.TileContext(nc))

        singles = ctx.enter_context(tc.tile_pool(name="singles", bufs=1))
        xpool = ctx.enter_context(tc.tile_pool(name="xin", bufs=6))
        sqpool = ctx.enter_context(tc.tile_pool(name="sq", bufs=2))
        opool = ctx.enter_context(tc.tile_pool(name="oout", bufs=3))
        stats = ctx.enter_context(tc.tile_pool(name="stats", bufs=4))
        psum_o = ctx.enter_context(tc.tile_pool(name="psum_o", bufs=4, space="PSUM"))
        psum_s = ctx.enter_context(tc.tile_pool(name="psum_s", bufs=2, space="PSUM"))
        psum_t = ctx.enter_context(tc.tile_pool(name="psum_t", bufs=2, space="PSUM"))

        zero_bias = singles.tile([P, 1], F32)
        nc.vector.memset(zero_bias, 0.0)
        ones_f = singles.tile([P, 1], F32)
        nc.vector.memset(ones_f, 1.0)
        ones_k = singles.tile([P, 1], F32R)   # reduce-over-partitions stationary
        nc.vector.tensor_copy(out=ones_k, in_=ones_f)
        eps_bias = singles.tile([P, 1], F32)
        nc.vector.memset(eps_bias, EPS)
        # ---- W normalization (once; scalar queue so x0 owns sync) ----
        w_sb = singles.tile([P, KC, OUT_DIM], F32)
        nc.scalar.dma_start(w_sb, w_d)
        eye4 = singles.tile([JT, JT], F32)    # transpose moving operand
        nc.scalar.dma_start(eye4, eye_d)
        wsq = singles.tile([P, KC, OUT_DIM], F32)  # scratch squares
        wssq = singles.tile([P, KC], F32)
        for c in range(KC):
            nc.scalar.activation(
                out=wsq[:, c, :],
                in_=w_sb[:, c, :],
                func=mybir.ActivationFunctionType.Square,
                bias=zero_bias,
                accum_out=wssq[:, c : c + 1],
            )
        nc.vector.tensor_scalar_max(wssq, wssq, EPS)
        nc.scalar.activation(
            out=wssq, in_=wssq, func=mybir.ActivationFunctionType.Sqrt, bias=zero_bias
        )
        wrs = singles.tile([P, KC], F32)
        nc.vector.reciprocal(wrs, wssq)
        # f32r so the PE matmul runs at 1 cycle/row; producer rounds to f32r
        wn_sb = singles.tile([P, KC, OUT_DIM], F32R)
        for c in range(KC):
            nc.vector.tensor_scalar_mul(wn_sb[:, c, :], w_sb[:, c, :], wrs[:, c : c + 1])

        x_tiles = {}
        s_mid = {}
        s_cols = {}

        def emit_xdma(g):
            x_sb = xpool.tile([P, KC, G], F32R)
            nc.sync.dma_start(x_sb, xt_d[:, ds(g * KC * G, KC * G)])
            x_tiles[g] = x_sb

        def emit_schain_a(g):
            """squares + partition reduce + evict + shuffle DMA."""
            x_sb = x_tiles[g]
            xsq = sqpool.tile([P, KC, G], F32R)
            nc.scalar.activation(
                out=xsq[:, 0:2, :],
                in_=x_sb[:, 0:2, :],
                func=mybir.ActivationFunctionType.Square,
                bias=zero_bias,
            )
            nc.vector.tensor_mul(xsq[:, 2:4, :], x_sb[:, 2:4, :], x_sb[:, 2:4, :])
            ps_ssq = psum_s.tile([1, G], F32)
            for c in range(KC):
                nc.tensor.matmul(
                    ps_ssq,
                    lhsT=ones_k[:, :],
                    rhs=xsq[:, c, :],
                    start=(c == 0),
                    stop=(c == KC - 1),
                )
            s_row = stats.tile([1, G], F32)
            nc.vector.tensor_copy(out=s_row, in_=ps_ssq)
            s4 = stats.tile([JT, P], F32)
            nc.scalar.dma_start(s4, s_row)
            s_mid[g] = s4

        def emit_schain_b(g):
            """eye4 transpose matmul + sqrt + reciprocal."""
            s4 = s_mid.pop(g)
            ps_s = psum_t.tile([P, JT], F32)
            nc.tensor.matmul(ps_s, lhsT=s4, rhs=eye4)
            sq_s = stats.tile([P, JT], F32)
            nc.scalar.activation(
                out=sq_s,
                in_=ps_s,
                func=mybir.ActivationFunctionType.Sqrt,
                bias=eps_bias,
            )
            s_col = stats.tile([P, JT], F32)
            nc.vector.reciprocal(s_col, sq_s)
            s_cols[g] = s_col

        def emit_gemm_j(g, j, ot, s_col):
            po = psum_o.tile([P, OUT_DIM], F32)
            x_sb = x_tiles[g]
            for c in range(KC):
                nc.tensor.matmul(
                    po,
                    lhsT=x_sb[:, c, ds(j * P, P)],
                    rhs=wn_sb[:, c, :],
                    start=(c == 0),
                    stop=(c == KC - 1),
                )
            # fused normalize: per-partition scale while evicting PSUM;
            # engine alternates per group to balance ACT/DVE
            if g % 2 == 0:
                nc.scalar.activation(
                    out=ot[:, j, :],
                    in_=po,
                    func=mybir.ActivationFunctionType.Copy,
                    scale=s_col[:, j : j + 1],
                )
            else:
                nc.vector.tensor_scalar_mul(ot[:, j, :], po, s_col[:, j : j + 1])

        # ---- prologue ----
        for g in range(min(4, N_GROUPS)):
            emit_xdma(g)
        emit_schain_a(0)
        emit_schain_a(1)
        emit_schain_b(0)

        # ---- steady-state: GEMM(g) with s_col(g) precomputed; s-chain
        # runs two groups ahead (a: g+2, b: g+1) ----
        for g in range(N_GROUPS):
            s_col = s_cols.pop(g)
            if g + 4 < N_GROUPS:
                emit_xdma(g + 4)
            ot = opool.tile([P, JT, OUT_DIM], F32)
            emit_gemm_j(g, 0, ot, s_col)
            emit_gemm_j(g, 1, ot, s_col)
            if g + 2 < N_GROUPS:
                emit_schain_a(g + 2)
            emit_gemm_j(g, 2, ot, s_col)
            emit_gemm_j(g, 3, ot, s_col)
            if g + 1 < N_GROUPS:
                emit_schain_b(g + 1)
            # store: DRAM row = g*512 + 4p + j -> one 7392B line/partition
            dst = bass.AP(
                tensor=o_d.tensor,
                offset=g * G * OUT_DIM,
                ap=[[JT * OUT_DIM, P], [OUT_DIM, JT], [1, OUT_DIM]],
            )
            nc.gpsimd.dma_start(dst, ot)
            del x_tiles[g]

    nc.compile()
    return nc


_NC_CACHE = None
LAST_RESULTS = None  # BassKernelResults of the most recent run (for profiling)

# within each 512-row window: local column i <-> global row 4*(i%128) + i//128
_PERM = 4 * (np.arange(G) % P) + np.arange(G) // P


def kernel(x: np.ndarray, W: np.ndarray) -> np.ndarray:
    global _NC_CACHE, LAST_RESULTS
    if _NC_CACHE is None:
        _NC_CACHE = _build_bass()
    nc = _NC_CACHE

    x = np.asarray(x, dtype=np.float32)
    W = np.asarray(W, dtype=np.float32)
    wt = np.ascontiguousarray(
        W.reshape(KC, P, OUT_DIM).transpose(1, 0, 2).reshape(P, KC * OUT_DIM)
    )
    cols = np.arange(N_GROUPS)[:, None] * G + _PERM[None, :]   # [16, 512]
    in_maps = []
    for i in range(N_CORES):
        sT = x[i * B_PER : (i + 1) * B_PER].T                  # [512, 8192]
        tmp = sT[:, cols]                                      # [512, 16, 512]
        tmp = tmp.reshape(KC, P, N_GROUPS, G).transpose(1, 2, 0, 3)
        xt = np.ascontiguousarray(tmp.reshape(P, N_GROUPS * KC * G))
        in_maps.append({"xt": xt, "w": wt, "eye4": np.eye(JT, dtype=np.float32)})
    res = bass_utils.run_bass_kernel_spmd(nc, in_maps, core_ids=list(range(N_CORES)))
    LAST_RESULTS = res
    out = np.concatenate([np.asarray(r["o"]) for r in res.results], axis=0)
    return out
